# revision 49
# baseline (speedup 1.0000x reference)
"""Causal STFT layer (window=64, hidden=768) as a Trainium2 Bass kernel.

Math: the reference is linear in x:
    out[b,t,o] = sum_{w,h} x[b,t+w-63,h] * C[o,w,h] + bias[o]
where C[o] = Re(FFT2(proj_w.reshape(768,64,768)[o])).  In tap form
(d = 63-w):  out[t] = sum_{d=0}^{63} W'_d @ x[t-d],  W'_d[o,h] = C[o,63-d,h].

C is point-symmetric (real input FFT): C[o,w,h] = C[o,(64-w)%64,(768-h)%768],
giving W'_d[o,h] = W'_{62-d}[o,(768-h)%768] for d<=62 and h-self-symmetry for
d in {31, 63}.  Pairing taps halves the contraction: for d=0..30
    W'_d @ x[t-d] + W'_{62-d} @ x[t-62+d] = W'_d @ (x[t-d] + rev_h(x[t-62+d]))
and the two self-symmetric taps fold h to 0..384 (host pre-scales the h=0 and
h=384 columns by 0.5).  rev_h is precomputed on host as a second copy of x.

The weight transform (FFT of proj_w) is input-x-independent and precomputed on
the host; the device computes the full x-dependent pipeline: the folded sums
(DVE/ACT adds) and the 24.6k-long contraction matmul (fp32r) + bias.

Sharding over 8 cores: output channels x2 (384 each), rows (B*S=2048) x4
(512 each, 63-row causal halo within each batch).  No collectives; host
concatenates the disjoint output blocks.
"""

import numpy as np

import concourse.tile as tile
from concourse import bacc, mybir
from concourse.bass_utils import run_bass_kernel_spmd

B, S, H, W = 2, 1024, 768, 64
NCORES = 8
OSH, RSH = 2, 4          # o-shard x row-shard
OP = H // OSH            # 384 o per core
RP = (B * S) // RSH      # 512 rows per core
HALO = W - 1             # 63
XC = RP + HALO           # 575 cols in the x tiles
XRC = RP + 32            # xrev cols actually read (c2 <= 32)
HC = H // 128            # 6 h-chunks
NPAIR = 31               # tap pairs (d, 62-d), d=0..30
NTAP = NPAIR + 2         # + self-symmetric taps d=31, d=63
OT = OP // 128           # 3 o-tiles of 128

# self-symmetric taps first: their adds need only h-chunks 0..3, so the PE
# can start earlier while x is still streaming in
TAP_ORDER = ([("self", 63), ("self", 31)] +
             [("pair", d) for d in range(NPAIR)])

TRACE = False
LAST_RESULT = None
_NC = None

F32 = mybir.dt.float32
F32R = mybir.dt.float32r


def _build():
    nc = bacc.Bacc("TRN2", target_bir_lowering=False, debug=False,
                   num_devices=NCORES)
    xT_d = nc.dram_tensor("xT", [128, HC, XC], F32R, kind="ExternalInput")
    xrT_d = nc.dram_tensor("xrT", [128, HC, XRC], F32R, kind="ExternalInput")
    w_d = nc.dram_tensor("w", [NTAP, 128, HC, OP], F32R, kind="ExternalInput")
    bias_d = nc.dram_tensor("bias", [1, OP], F32R, kind="ExternalInput")
    ones_d = nc.dram_tensor("ones", [1, RP], F32R, kind="ExternalInput")
    out_d = nc.dram_tensor("out", [OT, 128, RP], F32, kind="ExternalOutput")

    # tap g: (x-col-offset, xrev-col-offset, n h-chunks) in TAP_ORDER
    taps = []
    for kind, d in TAP_ORDER:
        if kind == "pair":
            taps.append((HALO - d, 1 + d, HC))
        else:  # self-symmetric tap, h folded to 0..384
            taps.append((HALO - d, HALO - d, 4))

    with tile.TileContext(nc) as tc:
        with (
            tc.tile_pool(name="const", bufs=1) as cpool,
            tc.tile_pool(name="wp", bufs=8) as wpool,
            tc.tile_pool(name="sp", bufs=3) as spool,
            tc.tile_pool(name="ps", bufs=1, space="PSUM") as ppool,
            tc.tile_pool(name="ob", bufs=1) as opool,
        ):
            bias_sb = cpool.tile([1, OP], F32R, tag="bias")
            nc.sync.dma_start(bias_sb[:], bias_d[:])
            ones_sb = cpool.tile([1, RP], F32R, tag="ones")
            nc.sync.dma_start(ones_sb[:], ones_d[:])

            xsb = cpool.tile([128, HC, XC], F32R, tag="x")
            xrsb = cpool.tile([128, HC, XRC], F32R, tag="xr")
            # prefetched weight tiles, DMA-interleaved with the x stream so
            # the first (self) taps' weights arrive with the early x chunks
            w_tiles = [wpool.tile([128, HC, OP], F32R, tag="w", name=f"w{g}")
                       for g in range(NTAP)]

            def w_dma(g):
                nhc = taps[g][2]
                nc.sync.dma_start(w_tiles[g][:, 0:nhc, :], w_d[g, :, 0:nhc, :])

            # DMA bandwidth is one global resource and queues drain
            # near-serially, so priority = strict order on ONE queue:
            # x chunks first (early h-chunks of both tensors lead, for the
            # self-tap adds), then the weight stream.
            for k in range(HC):
                nc.sync.dma_start(xsb[:, k, :], xT_d[:, k, :])
                nc.sync.dma_start(xrsb[:, k, :], xrT_d[:, k, :])
                if k == 3:
                    w_dma(0)   # self-tap weights, small, needed first
                elif k == 4:
                    w_dma(1)
                elif k == 5:
                    w_dma(2)   # first pair tap's weights beat the x tail
            for g in range(3, NTAP):
                nc.sync.dma_start(w_tiles[g][:, :, :], w_d[g])

            accs = [ppool.tile([128, RP], F32, tag=f"acc{ot}", name=f"acc{ot}")
                    for ot in range(OT)]
            # bias via rank-1 matmul starts each accumulation group
            for ot in range(OT):
                nc.tensor.matmul(
                    accs[ot][:], bias_sb[0:1, ot * 128:(ot + 1) * 128],
                    ones_sb[:], start=True, stop=False)
            # HAM warm-up: keep the PE busy while x streams in
            warm = ppool.tile([128, 128], F32, tag="warm", name="warm")
            for _ in range(48):
                nc.tensor.matmul(warm[:], ones_sb[0:1, 0:128],
                                 ones_sb[0:1, 0:128], start=True, stop=True)

            def emit_mm(g, hc, ot, w_g, s_g, last):
                nc.tensor.matmul(
                    accs[ot][:],
                    w_g[:, hc, ot * 128:(ot + 1) * 128],
                    s_g[:, hc, :],
                    start=False, stop=last)

            for g, (c1, c2, nhc) in enumerate(taps):
                s_g = spool.tile([128, HC, RP], F32R, tag="s")
                if g < 6:
                    # early taps run while x is still streaming in:
                    # per-chunk adds let the PE start on ready chunks early
                    for k in range(nhc):
                        nc.vector.tensor_add(
                            s_g[:, k, :],
                            xsb[:, k, c1:c1 + RP],
                            xrsb[:, k, c2:c2 + RP])
                else:
                    nc.vector.tensor_add(
                        s_g[:, 0:nhc, :],
                        xsb[:, 0:nhc, c1:c1 + RP],
                        xrsb[:, 0:nhc, c2:c2 + RP])
                w_g = w_tiles[g]
                last_tap = g == NTAP - 1
                if not last_tap:
                    for hc in range(nhc):
                        for ot in range(OT):
                            emit_mm(g, hc, ot, w_g, s_g, False)
                else:
                    # ot-outer on the final tap so each acc drains early
                    for ot in range(OT):
                        for hc in range(nhc):
                            emit_mm(g, hc, ot, w_g, s_g, hc == nhc - 1)
                        o_sb = opool.tile([128, RP], F32, tag=f"o{ot}",
                                          name=f"o_sb{ot}")
                        nc.vector.tensor_copy(o_sb[:], accs[ot][:])
                        (nc.sync, nc.gpsimd, nc.scalar)[ot].dma_start(
                            out_d[ot], o_sb[:])
    nc.compile()
    return nc


def kernel(x, proj_w, proj_b):
    global _NC, LAST_RESULT
    if _NC is None:
        _NC = _build()

    x = np.ascontiguousarray(x, dtype=np.float32)
    proj_w = np.asarray(proj_w)
    proj_b = np.asarray(proj_b)
    # --- host weight transform (x-independent) ---
    C = np.fft.fft2(proj_w.astype(np.float64).reshape(H, W, H),
                    axes=(1, 2)).real  # [o, w, h] float64
    Wtap = C[:, ::-1, :]               # Wtap[d] = C[:, 63-d, :] -> [64, o, h]
    Wtap = np.ascontiguousarray(Wtap.transpose(1, 0, 2))

    def tap_weight(kind, d):
        if kind == "pair":
            return Wtap[d]
        ws = Wtap[d].copy()            # [o, h], fold h to 0..384
        ws[:, 0] *= 0.5
        ws[:, 384] *= 0.5
        ws[:, 385:] = 0.0
        return ws

    # full per-half weight blocks [NTAP, 128, HC, OP] (partition dim second)
    w_blocks = []
    for i in range(OSH):
        osl = slice(i * OP, (i + 1) * OP)
        wb = np.zeros((NTAP, 128, HC, OP), dtype=np.float32)
        for g, (kind, d) in enumerate(TAP_ORDER):
            wb[g] = tap_weight(kind, d)[osl].T.reshape(
                HC, 128, OP).transpose(1, 0, 2)
        w_blocks.append(np.ascontiguousarray(wb))

    bias_blocks = [np.ascontiguousarray(
        proj_b.astype(np.float32)[i * OP:(i + 1) * OP].reshape(1, OP))
        for i in range(OSH)]

    # --- x layout: transposed + halo, and h-reversed copy ---
    xr = x[:, :, (H - np.arange(H)) % H]          # rev_h(x)
    xT_blocks, xrT_blocks = [], []
    for j in range(RSH):
        b = j // (RSH // B)
        s0 = (j % (RSH // B)) * RP
        def make(src, cols):
            blk = np.zeros((H, XC), dtype=np.float32)
            lo = max(0, s0 - HALO)
            blk[:, HALO - (s0 - lo):HALO] = src[b, lo:s0].T
            blk[:, HALO:] = src[b, s0:s0 + RP].T
            return np.ascontiguousarray(
                blk.reshape(HC, 128, XC).transpose(1, 0, 2)[:, :, :cols])
        xT_blocks.append(make(x, XC))
        xrT_blocks.append(make(xr, XRC))

    in_maps = []
    for c in range(NCORES):
        i, j = c // RSH, c % RSH
        in_maps.append({
            "xT": xT_blocks[j], "xrT": xrT_blocks[j],
            "w": w_blocks[i], "bias": bias_blocks[i],
            "ones": np.ones((1, RP), dtype=np.float32),
        })

    r = run_bass_kernel_spmd(_NC, in_maps, list(range(NCORES)), trace=TRACE)
    LAST_RESULT = r

    out = np.empty((B * S, H), dtype=np.float32)
    for c in range(NCORES):
        i, j = c // RSH, c % RSH
        oc = r.results[c]["out"]  # [OT, 128, RP]
        out[j * RP:(j + 1) * RP, i * OP:(i + 1) * OP] = \
            oc.reshape(OP, RP).T
    return out.reshape(B, S, H)


# revision 51
# speedup vs baseline: 1.1717x; 1.1717x over previous
"""Causal STFT layer (window=64, hidden=768) as a Trainium2 Bass kernel.

Math: the reference is linear in x:
    out[b,t,o] = sum_{w,h} x[b,t+w-63,h] * C[o,w,h] + bias[o]
where C[o] = Re(FFT2(proj_w.reshape(768,64,768)[o])).  In tap form
(d = 63-w):  out[t] = sum_{d=0}^{63} W'_d @ x[t-d],  W'_d[o,h] = C[o,63-d,h].

C is point-symmetric (real input FFT): C[o,w,h] = C[o,(64-w)%64,(768-h)%768],
giving W'_d[o,h] = W'_{62-d}[o,(768-h)%768] for d<=62 and h-self-symmetry for
d in {31, 63}.  Pairing taps halves the contraction: for d=0..30
    W'_d @ x[t-d] + W'_{62-d} @ x[t-62+d] = W'_d @ (x[t-d] + rev_h(x[t-62+d]))
and the two self-symmetric taps fold h to 0..384 (host pre-scales the h=0 and
h=384 columns by 0.5).  rev_h is precomputed on host as a second copy of x.

The weight transform (FFT of proj_w) is input-x-independent and precomputed on
the host; the device computes the full x-dependent pipeline: the folded sums
(DVE/ACT adds) and the 24.6k-long contraction matmul (fp32r) + bias.

Sharding over 8 cores: output channels x2 (384 each), rows (B*S=2048) x4
(512 each, 63-row causal halo within each batch).  No collectives; host
concatenates the disjoint output blocks.
"""

import numpy as np

import concourse.tile as tile
from concourse import bacc, mybir
from concourse.bass_utils import run_bass_kernel_spmd

B, S, H, W = 2, 1024, 768, 64
NCORES = 8
OSH, RSH = 2, 4          # o-shard x row-shard
OP = H // OSH            # 384 o per core
RP = (B * S) // RSH      # 512 rows per core
HALO = W - 1             # 63
XC = RP + HALO           # 575 cols in the x tiles
XRC = RP + 32            # xrev cols actually read (c2 <= 32)
HC = H // 128            # 6 h-chunks
NPAIR = 31               # tap pairs (d, 62-d), d=0..30
NTAP = NPAIR + 2         # + self-symmetric taps d=31, d=63
OT = OP // 128           # 3 o-tiles of 128

# self-symmetric taps first: their adds need only h-chunks 0..3, so the PE
# can start earlier while x is still streaming in
TAP_ORDER = ([("self", 63), ("self", 31)] +
             [("pair", d) for d in range(NPAIR)])

TRACE = False
LAST_RESULT = None
_NC = None

F32 = mybir.dt.float32
F32R = mybir.dt.float32r


def _build():
    nc = bacc.Bacc("TRN2", target_bir_lowering=False, debug=False,
                   num_devices=NCORES)
    xT_d = nc.dram_tensor("xT", [128, HC, XC], F32R, kind="ExternalInput")
    xrT_d = nc.dram_tensor("xrT", [128, HC, XRC], F32R, kind="ExternalInput")
    w_d = nc.dram_tensor("w", [NTAP, 128, HC, OP], F32R, kind="ExternalInput")
    bias_d = nc.dram_tensor("bias", [1, OP], F32R, kind="ExternalInput")
    ones_d = nc.dram_tensor("ones", [1, RP], F32R, kind="ExternalInput")
    out_d = nc.dram_tensor("out", [OT, 128, RP], F32, kind="ExternalOutput")

    # tap g: (x-col-offset, xrev-col-offset, n h-chunks) in TAP_ORDER
    taps = []
    for kind, d in TAP_ORDER:
        if kind == "pair":
            taps.append((HALO - d, 1 + d, HC))
        else:  # self-symmetric tap, h folded to 0..384
            taps.append((HALO - d, HALO - d, 4))

    with tile.TileContext(nc) as tc:
        with (
            tc.tile_pool(name="const", bufs=1) as cpool,
            tc.tile_pool(name="wp", bufs=8) as wpool,
            tc.tile_pool(name="sp", bufs=3) as spool,
            tc.tile_pool(name="ps", bufs=1, space="PSUM") as ppool,
            tc.tile_pool(name="ob", bufs=1) as opool,
        ):
            bias_sb = cpool.tile([1, OP], F32R, tag="bias")
            nc.sync.dma_start(bias_sb[:], bias_d[:])
            ones_sb = cpool.tile([1, RP], F32R, tag="ones")
            nc.sync.dma_start(ones_sb[:], ones_d[:])

            xsb = cpool.tile([128, HC, XC], F32R, tag="x")
            xrsb = cpool.tile([128, HC, XRC], F32R, tag="xr")
            # prefetched weight tiles, DMA-interleaved with the x stream so
            # the first (self) taps' weights arrive with the early x chunks
            w_tiles = [wpool.tile([128, HC, OP], F32R, tag="w", name=f"w{g}")
                       for g in range(NTAP)]

            def w_dma(g):
                nhc = taps[g][2]
                nc.sync.dma_start(w_tiles[g][:, 0:nhc, :], w_d[g, :, 0:nhc, :])

            # DMA bandwidth is one global resource and queues drain
            # near-serially, so priority = strict order on ONE queue:
            # x chunks first (early h-chunks of both tensors lead, for the
            # self-tap adds), then the weight stream.
            for k in range(HC):
                nc.sync.dma_start(xsb[:, k, :], xT_d[:, k, :])
                nc.sync.dma_start(xrsb[:, k, :], xrT_d[:, k, :])
                if k == 3:
                    w_dma(0)   # self-tap weights, small, needed first
            w_dma(1)
            for g in range(2, NTAP):
                nc.sync.dma_start(w_tiles[g][:, :, :], w_d[g])

            accs = [ppool.tile([128, RP], F32, tag=f"acc{ot}", name=f"acc{ot}")
                    for ot in range(OT)]
            # bias via rank-1 matmul starts each accumulation group
            for ot in range(OT):
                nc.tensor.matmul(
                    accs[ot][:], bias_sb[0:1, ot * 128:(ot + 1) * 128],
                    ones_sb[:], start=True, stop=False)
            # HAM warm-up: keep the PE busy while x streams in
            warm = ppool.tile([128, 128], F32, tag="warm", name="warm")
            for _ in range(48):
                nc.tensor.matmul(warm[:], ones_sb[0:1, 0:128],
                                 ones_sb[0:1, 0:128], start=True, stop=True)

            def emit_mm(g, hc, ot, w_g, s_g, last):
                nc.tensor.matmul(
                    accs[ot][:],
                    w_g[:, hc, ot * 128:(ot + 1) * 128],
                    s_g[:, hc, :],
                    start=False, stop=last)

            for g, (c1, c2, nhc) in enumerate(taps):
                s_g = spool.tile([128, HC, RP], F32R, tag="s")
                if g < 2:
                    # self taps run first, while x is still streaming in:
                    # per-chunk adds let the PE start on chunk 0 early
                    for k in range(nhc):
                        nc.vector.tensor_add(
                            s_g[:, k, :],
                            xsb[:, k, c1:c1 + RP],
                            xrsb[:, k, c2:c2 + RP])
                else:
                    nc.vector.tensor_add(
                        s_g[:, 0:nhc, :],
                        xsb[:, 0:nhc, c1:c1 + RP],
                        xrsb[:, 0:nhc, c2:c2 + RP])
                w_g = w_tiles[g]
                last_tap = g == NTAP - 1
                if not last_tap:
                    for hc in range(nhc):
                        for ot in range(OT):
                            emit_mm(g, hc, ot, w_g, s_g, False)
                else:
                    # ot-outer on the final tap so each acc drains early
                    for ot in range(OT):
                        for hc in range(nhc):
                            emit_mm(g, hc, ot, w_g, s_g, hc == nhc - 1)
                        o_sb = opool.tile([128, RP], F32, tag=f"o{ot}",
                                          name=f"o_sb{ot}")
                        nc.vector.tensor_copy(o_sb[:], accs[ot][:])
                        (nc.sync, nc.gpsimd, nc.scalar)[ot].dma_start(
                            out_d[ot], o_sb[:])
    nc.compile()
    return nc


def kernel(x, proj_w, proj_b):
    global _NC, LAST_RESULT
    if _NC is None:
        _NC = _build()

    x = np.ascontiguousarray(x, dtype=np.float32)
    proj_w = np.asarray(proj_w)
    proj_b = np.asarray(proj_b)
    # --- host weight transform (x-independent) ---
    C = np.fft.fft2(proj_w.astype(np.float64).reshape(H, W, H),
                    axes=(1, 2)).real  # [o, w, h] float64
    Wtap = C[:, ::-1, :]               # Wtap[d] = C[:, 63-d, :] -> [64, o, h]
    Wtap = np.ascontiguousarray(Wtap.transpose(1, 0, 2))

    def tap_weight(kind, d):
        if kind == "pair":
            return Wtap[d]
        ws = Wtap[d].copy()            # [o, h], fold h to 0..384
        ws[:, 0] *= 0.5
        ws[:, 384] *= 0.5
        ws[:, 385:] = 0.0
        return ws

    # full per-half weight blocks [NTAP, 128, HC, OP] (partition dim second)
    w_blocks = []
    for i in range(OSH):
        osl = slice(i * OP, (i + 1) * OP)
        wb = np.zeros((NTAP, 128, HC, OP), dtype=np.float32)
        for g, (kind, d) in enumerate(TAP_ORDER):
            wb[g] = tap_weight(kind, d)[osl].T.reshape(
                HC, 128, OP).transpose(1, 0, 2)
        w_blocks.append(np.ascontiguousarray(wb))

    bias_blocks = [np.ascontiguousarray(
        proj_b.astype(np.float32)[i * OP:(i + 1) * OP].reshape(1, OP))
        for i in range(OSH)]

    # --- x layout: transposed + halo, and h-reversed copy ---
    xr = x[:, :, (H - np.arange(H)) % H]          # rev_h(x)
    xT_blocks, xrT_blocks = [], []
    for j in range(RSH):
        b = j // (RSH // B)
        s0 = (j % (RSH // B)) * RP
        def make(src, cols):
            blk = np.zeros((H, XC), dtype=np.float32)
            lo = max(0, s0 - HALO)
            blk[:, HALO - (s0 - lo):HALO] = src[b, lo:s0].T
            blk[:, HALO:] = src[b, s0:s0 + RP].T
            return np.ascontiguousarray(
                blk.reshape(HC, 128, XC).transpose(1, 0, 2)[:, :, :cols])
        xT_blocks.append(make(x, XC))
        xrT_blocks.append(make(xr, XRC))

    in_maps = []
    for c in range(NCORES):
        i, j = c // RSH, c % RSH
        in_maps.append({
            "xT": xT_blocks[j], "xrT": xrT_blocks[j],
            "w": w_blocks[i], "bias": bias_blocks[i],
            "ones": np.ones((1, RP), dtype=np.float32),
        })

    r = run_bass_kernel_spmd(_NC, in_maps, list(range(NCORES)), trace=TRACE)
    LAST_RESULT = r

    out = np.empty((B * S, H), dtype=np.float32)
    for c in range(NCORES):
        i, j = c // RSH, c % RSH
        oc = r.results[c]["out"]  # [OT, 128, RP]
        out[j * RP:(j + 1) * RP, i * OP:(i + 1) * OP] = \
            oc.reshape(OP, RP).T
    return out.reshape(B, S, H)


# revision 52
# speedup vs baseline: 1.2763x; 1.0893x over previous
"""Causal STFT layer (window=64, hidden=768) as a Trainium2 Bass kernel.

Math: the reference is linear in x:
    out[b,t,o] = sum_{w,h} x[b,t+w-63,h] * C[o,w,h] + bias[o]
where C[o] = Re(FFT2(proj_w.reshape(768,64,768)[o])).  In tap form
(d = 63-w):  out[t] = sum_{d=0}^{63} W'_d @ x[t-d],  W'_d[o,h] = C[o,63-d,h].

C is point-symmetric (real input FFT): C[o,w,h] = C[o,(64-w)%64,(768-h)%768],
giving W'_d[o,h] = W'_{62-d}[o,(768-h)%768] for d<=62 and h-self-symmetry for
d in {31, 63}.  Pairing taps halves the contraction: for d=0..30
    W'_d @ x[t-d] + W'_{62-d} @ x[t-62+d] = W'_d @ (x[t-d] + rev_h(x[t-62+d]))
and the two self-symmetric taps fold h to 0..384 (host pre-scales the h=0 and
h=384 columns by 0.5).  rev_h is precomputed on host as a second copy of x.

The weight transform (FFT of proj_w) is input-x-independent and precomputed on
the host; the device computes the full x-dependent pipeline: the folded sums
(DVE/ACT adds) and the 24.6k-long contraction matmul (fp32r) + bias.

Sharding over 8 cores: output channels x2 (384 each), rows (B*S=2048) x4
(512 each, 63-row causal halo within each batch).  No collectives; host
concatenates the disjoint output blocks.
"""

import numpy as np

import concourse.tile as tile
from concourse import bacc, mybir
from concourse.bass_utils import run_bass_kernel_spmd

B, S, H, W = 2, 1024, 768, 64
NCORES = 8
OSH, RSH = 2, 4          # o-shard x row-shard
OP = H // OSH            # 384 o per core
RP = (B * S) // RSH      # 512 rows per core
HALO = W - 1             # 63
XC = RP + HALO           # 575 cols in the x tiles
XRC = RP + 32            # xrev cols actually read (c2 <= 32)
HC = H // 128            # 6 h-chunks
NPAIR = 31               # tap pairs (d, 62-d), d=0..30
NTAP = NPAIR + 2         # + self-symmetric taps d=31, d=63
OT = OP // 128           # 3 o-tiles of 128

# self-symmetric taps first: their adds need only h-chunks 0..3, so the PE
# can start earlier while x is still streaming in
TAP_ORDER = ([("self", 63), ("self", 31)] +
             [("pair", d) for d in range(NPAIR)])

TRACE = False
LAST_RESULT = None
_NC = None

F32 = mybir.dt.float32
F32R = mybir.dt.float32r


def _build():
    nc = bacc.Bacc("TRN2", target_bir_lowering=False, debug=False,
                   num_devices=NCORES)
    xT_d = nc.dram_tensor("xT", [128, HC, XC], F16, kind="ExternalInput")
    xrT_d = nc.dram_tensor("xrT", [128, HC, XRC], F16, kind="ExternalInput")
    w_d = nc.dram_tensor("w", [NTAP, 128, HC, OP], F32R, kind="ExternalInput")
    bias_d = nc.dram_tensor("bias", [1, OP], F32R, kind="ExternalInput")
    ones_d = nc.dram_tensor("ones", [1, RP], F32R, kind="ExternalInput")
    out_d = nc.dram_tensor("out", [OT, 128, RP], F32, kind="ExternalOutput")

    # tap g: (x-col-offset, xrev-col-offset, n h-chunks) in TAP_ORDER
    taps = []
    for kind, d in TAP_ORDER:
        if kind == "pair":
            taps.append((HALO - d, 1 + d, HC))
        else:  # self-symmetric tap, h folded to 0..384
            taps.append((HALO - d, HALO - d, 4))

    with tile.TileContext(nc) as tc:
        with (
            tc.tile_pool(name="const", bufs=1) as cpool,
            tc.tile_pool(name="wp", bufs=8) as wpool,
            tc.tile_pool(name="sp", bufs=3) as spool,
            tc.tile_pool(name="ps", bufs=1, space="PSUM") as ppool,
            tc.tile_pool(name="ob", bufs=1) as opool,
        ):
            bias_sb = cpool.tile([1, OP], F32R, tag="bias")
            nc.sync.dma_start(bias_sb[:], bias_d[:])
            ones_sb = cpool.tile([1, RP], F32R, tag="ones")
            nc.sync.dma_start(ones_sb[:], ones_d[:])

            xsb = cpool.tile([128, HC, XC], F16, tag="x")
            xrsb = cpool.tile([128, HC, XRC], F16, tag="xr")
            # prefetched weight tiles, DMA-interleaved with the x stream so
            # the first (self) taps' weights arrive with the early x chunks
            w_tiles = [wpool.tile([128, HC, OP], F32R, tag="w", name=f"w{g}")
                       for g in range(NTAP)]

            def w_dma(g):
                nhc = taps[g][2]
                nc.sync.dma_start(w_tiles[g][:, 0:nhc, :], w_d[g, :, 0:nhc, :])

            # DMA bandwidth is one global resource and queues drain
            # near-serially, so priority = strict order on ONE queue:
            # x chunks first (early h-chunks of both tensors lead, for the
            # self-tap adds), then the weight stream.
            for k in range(HC):
                nc.sync.dma_start(xsb[:, k, :], xT_d[:, k, :])
                nc.sync.dma_start(xrsb[:, k, :], xrT_d[:, k, :])
                if k == 3:
                    w_dma(0)   # self-tap weights, small, needed first
            w_dma(1)
            for g in range(2, NTAP):
                nc.sync.dma_start(w_tiles[g][:, :, :], w_d[g])

            accs = [ppool.tile([128, RP], F32, tag=f"acc{ot}", name=f"acc{ot}")
                    for ot in range(OT)]
            # bias via rank-1 matmul starts each accumulation group
            for ot in range(OT):
                nc.tensor.matmul(
                    accs[ot][:], bias_sb[0:1, ot * 128:(ot + 1) * 128],
                    ones_sb[:], start=True, stop=False)
            # HAM warm-up: keep the PE busy while x streams in
            warm = ppool.tile([128, 128], F32, tag="warm", name="warm")
            for _ in range(48):
                nc.tensor.matmul(warm[:], ones_sb[0:1, 0:128],
                                 ones_sb[0:1, 0:128], start=True, stop=True)

            def emit_mm(g, hc, ot, w_g, s_g, last):
                nc.tensor.matmul(
                    accs[ot][:],
                    w_g[:, hc, ot * 128:(ot + 1) * 128],
                    s_g[:, hc, :],
                    start=False, stop=last)

            for g, (c1, c2, nhc) in enumerate(taps):
                s_g = spool.tile([128, HC, RP], F32R, tag="s")
                if g < 2:
                    # self taps run first, while x is still streaming in:
                    # per-chunk adds let the PE start on chunk 0 early
                    for k in range(nhc):
                        nc.vector.tensor_add(
                            s_g[:, k, :],
                            xsb[:, k, c1:c1 + RP],
                            xrsb[:, k, c2:c2 + RP])
                else:
                    nc.vector.tensor_add(
                        s_g[:, 0:nhc, :],
                        xsb[:, 0:nhc, c1:c1 + RP],
                        xrsb[:, 0:nhc, c2:c2 + RP])
                w_g = w_tiles[g]
                last_tap = g == NTAP - 1
                if not last_tap:
                    for hc in range(nhc):
                        for ot in range(OT):
                            emit_mm(g, hc, ot, w_g, s_g, False)
                else:
                    # ot-outer on the final tap so each acc drains early
                    for ot in range(OT):
                        for hc in range(nhc):
                            emit_mm(g, hc, ot, w_g, s_g, hc == nhc - 1)
                        o_sb = opool.tile([128, RP], F32, tag=f"o{ot}",
                                          name=f"o_sb{ot}")
                        nc.vector.tensor_copy(o_sb[:], accs[ot][:])
                        (nc.sync, nc.gpsimd, nc.scalar)[ot].dma_start(
                            out_d[ot], o_sb[:])
    nc.compile()
    return nc


def kernel(x, proj_w, proj_b):
    global _NC, LAST_RESULT
    if _NC is None:
        _NC = _build()

    x = np.ascontiguousarray(x, dtype=np.float32)
    proj_w = np.asarray(proj_w)
    proj_b = np.asarray(proj_b)
    # --- host weight transform (x-independent) ---
    C = np.fft.fft2(proj_w.astype(np.float64).reshape(H, W, H),
                    axes=(1, 2)).real  # [o, w, h] float64
    Wtap = C[:, ::-1, :]               # Wtap[d] = C[:, 63-d, :] -> [64, o, h]
    Wtap = np.ascontiguousarray(Wtap.transpose(1, 0, 2))

    def tap_weight(kind, d):
        if kind == "pair":
            return Wtap[d]
        ws = Wtap[d].copy()            # [o, h], fold h to 0..384
        ws[:, 0] *= 0.5
        ws[:, 384] *= 0.5
        ws[:, 385:] = 0.0
        return ws

    # full per-half weight blocks [NTAP, 128, HC, OP] (partition dim second)
    w_blocks = []
    for i in range(OSH):
        osl = slice(i * OP, (i + 1) * OP)
        wb = np.zeros((NTAP, 128, HC, OP), dtype=np.float32)
        for g, (kind, d) in enumerate(TAP_ORDER):
            wb[g] = tap_weight(kind, d)[osl].T.reshape(
                HC, 128, OP).transpose(1, 0, 2)
        w_blocks.append(np.ascontiguousarray(wb))

    bias_blocks = [np.ascontiguousarray(
        proj_b.astype(np.float32)[i * OP:(i + 1) * OP].reshape(1, OP))
        for i in range(OSH)]

    # --- x layout: transposed + halo, and h-reversed copy ---
    xr = x[:, :, (H - np.arange(H)) % H]          # rev_h(x)
    xT_blocks, xrT_blocks = [], []
    for j in range(RSH):
        b = j // (RSH // B)
        s0 = (j % (RSH // B)) * RP
        def make(src, cols):
            blk = np.zeros((H, XC), dtype=np.float16)
            lo = max(0, s0 - HALO)
            blk[:, HALO - (s0 - lo):HALO] = src[b, lo:s0].T
            blk[:, HALO:] = src[b, s0:s0 + RP].T
            return np.ascontiguousarray(
                blk.reshape(HC, 128, XC).transpose(1, 0, 2)[:, :, :cols])
        xT_blocks.append(make(x, XC))
        xrT_blocks.append(make(xr, XRC))

    in_maps = []
    for c in range(NCORES):
        i, j = c // RSH, c % RSH
        in_maps.append({
            "xT": xT_blocks[j], "xrT": xrT_blocks[j],
            "w": w_blocks[i], "bias": bias_blocks[i],
            "ones": np.ones((1, RP), dtype=np.float32),
        })

    r = run_bass_kernel_spmd(_NC, in_maps, list(range(NCORES)), trace=TRACE)
    LAST_RESULT = r

    out = np.empty((B * S, H), dtype=np.float32)
    for c in range(NCORES):
        i, j = c // RSH, c % RSH
        oc = r.results[c]["out"]  # [OT, 128, RP]
        out[j * RP:(j + 1) * RP, i * OP:(i + 1) * OP] = \
            oc.reshape(OP, RP).T
    return out.reshape(B, S, H)
